# revision 3
# baseline (speedup 1.0000x reference)
"""Trainium2 Bass kernel for the Ablock spatial paradigm, v5 (fp16).

Reference computation (per sample, C=320 channels of 128x128):
    f    = silu(lem(x))
    fatt = lem(sigmoid(f) - 0.5)
    out  = (f + x) * fatt
where lem applies a per-channel circular 1-pixel shift S_c chosen by c%5:
    0: roll -1 along W   1: roll +1 along W
    2: roll -1 along H   3: roll +1 along H   4: identity

Using sigmoid(z)-0.5 = 0.5*tanh(z/2) and S commuting with elementwise ops:
    u = silu(x);  w = tanh(u/2)
    out = (S u + x) * (0.5 * S^2 w)
The kernel computes o2 = (S u + x) * (S^2 w); the host multiplies by 0.5
during the fp16->fp32 upconversion (exact).

Design (v5):
  - Whole pipeline in fp16 on device (harness gate 2e-2 rel; measured
    pipeline error ~1e-3): halves HBM traffic both ways and enables the
    DVE 2x packed mode (TensorTensor only - ScalarTensorPtr measured 1x
    on HW, so the 0.5 scale moved to the host).
  - ACT is the bottleneck (2 transcendental passes = 68us engine floor at
    1.2GHz): it runs exactly 2 clean full-tile activations per tile, with
    ALL shift handling on DVE input APs (which has slack at fp16 2x).
  - H-shift partition-boundary rows via small PE permutation matmuls into
    PSUM, consumed by DVE.
  - Loads on the sync HWDGE queue; stores on the GpSimd SWDGE queue; the
    permutation-matrix load on the scalar HWDGE queue (so tile 0's load
    is first in the sync queue).
  - Taper: the first and last tiles (W-shift / identity, row-splittable)
    are processed as two independent half-tiles, shortening the pipeline
    head (ACT starts after a half-size load) and tail (half-size final
    multiply + store).

Sharding: pure data-parallel, one batch sample per NeuronCore (B=8).
Layout: G=32 same-shift-type channels per tile [128, 4096] fp16; channel
k of a tile occupies partitions [4k, 4k+4); each partition holds 32
contiguous image rows (8KB). Tile load/store is one 3-dim DMA with 8KB
contiguous runs on both sides.
"""

import numpy as np

import concourse.bacc as bacc
import concourse.mybir as mybir
from concourse.bass_utils import run_bass_kernel_spmd
from concourse.tile import TileContext

B, C, H, W = 8, 320, 128, 128
G = 32  # channels of one shift type per tile
F = G * W  # tile free size (4096)
NTYPE_CH = C // 5  # channels per shift type (64)
NCHUNK = NTYPE_CH // G  # tiles per type (2)
PPC = H // G  # partitions per channel (4)
RPP = H // PPC  # image rows per partition (32)
FP32 = mybir.dt.float32
FP16 = mybir.dt.float16
AOP = mybir.AluOpType


def _emit(nc, tc, x_d, o_d, p_d):
    act = mybir.ActivationFunctionType
    with (
        tc.tile_pool(name="pp", bufs=1) as pp,
        tc.tile_pool(name="xp", bufs=5) as xp,
        tc.tile_pool(name="up", bufs=3) as up,
        tc.tile_pool(name="wp", bufs=3) as wp,
        tc.tile_pool(name="ap", bufs=3) as ap_,
        tc.tile_pool(name="op", bufs=5) as op_,
        tc.tile_pool(name="pup", bufs=2, space="PSUM") as pup,
        tc.tile_pool(name="pwp", bufs=2, space="PSUM") as pwp,
    ):
        # Block-diagonal permutation matrices (32 identical 4x4 circulant
        # blocks): as matmul stationary lhsT, (P_d.T @ v)[p,f] =
        # v[group(p)*4 + (p%4 + d) % 4, f] for d = +1 / -1. Loaded via the
        # scalar HWDGE queue to keep the sync queue free for tile loads.
        pm = pp.tile([H, 2 * H], FP16, name="pm")
        nc.scalar.dma_start(
            out=pm.rearrange("p (d i) -> p d i", d=2),
            in_=p_d.rearrange("d k i -> k d i"),
        )
        pm3 = pm.rearrange("p (d i) -> p d i", d=2)

        M = RPP

        def dram_ap(base, c0, m0, m1):
            ap = (
                base[c0 : c0 + 5 * (G - 1) + 1 : 5]
                .rearrange("k h w -> k (h w)")
                .rearrange("k (q f) -> k q f", f=RPP * W)
            )
            if (m0, m1) != (0, RPP):
                ap = ap[:, :, m0 * W : m1 * W]
            return ap

        def w_tile(r, c0, m0, m1):
            """One (possibly row-sliced) tile of a W-shift (r in {0,1}) or
            identity (r==4) type: rows m0..m1 of each partition."""
            ff = (m1 - m0) * W
            xt = xp.tile([H, ff], FP16, name="xt")
            nc.sync.dma_start(out=xt, in_=dram_ap(x_d, c0, m0, m1), single_packet=True)
            u = up.tile([H, ff], FP16, name="u")
            nc.scalar.activation(u, xt, act.Silu)
            w = wp.tile([H, ff], FP16, name="w")
            nc.scalar.activation(w, u, act.Tanh, scale=0.5)
            a = ap_.tile([H, ff], FP16, name="a")
            o = op_.tile([H, ff], FP16, name="o")
            x3 = xt.rearrange("p (m w) -> p m w", w=W)
            u3 = u.rearrange("p (m w) -> p m w", w=W)
            w3 = w.rearrange("p (m w) -> p m w", w=W)
            a3 = a.rearrange("p (m w) -> p m w", w=W)
            o3 = o.rearrange("p (m w) -> p m w", w=W)
            if r == 0:
                # S: out(.,j) = in(.,j+1) along W
                nc.vector.tensor_tensor(
                    a3[:, :, 0:127], u3[:, :, 1:128], x3[:, :, 0:127], AOP.add
                )
                nc.vector.tensor_tensor(
                    a3[:, :, 127:128], u3[:, :, 0:1], x3[:, :, 127:128], AOP.add
                )
                nc.vector.tensor_tensor(
                    o3[:, :, 0:126], w3[:, :, 2:128], a3[:, :, 0:126], AOP.mult
                )
                nc.vector.tensor_tensor(
                    o3[:, :, 126:128], w3[:, :, 0:2], a3[:, :, 126:128], AOP.mult
                )
            elif r == 1:
                # S: out(.,j) = in(.,j-1) along W
                nc.vector.tensor_tensor(
                    a3[:, :, 1:128], u3[:, :, 0:127], x3[:, :, 1:128], AOP.add
                )
                nc.vector.tensor_tensor(
                    a3[:, :, 0:1], u3[:, :, 127:128], x3[:, :, 0:1], AOP.add
                )
                nc.vector.tensor_tensor(
                    o3[:, :, 2:128], w3[:, :, 0:126], a3[:, :, 2:128], AOP.mult
                )
                nc.vector.tensor_tensor(
                    o3[:, :, 0:2], w3[:, :, 126:128], a3[:, :, 0:2], AOP.mult
                )
            else:
                nc.vector.tensor_tensor(a, u, xt, AOP.add)
                nc.vector.tensor_tensor(o, w, a, AOP.mult)
            nc.gpsimd.dma_start(out=dram_ap(o_d, c0, m0, m1), in_=o, single_packet=True)

        def h_tile(r, c0):
            """One full tile of an H-shift type (r in {2,3})."""
            xt = xp.tile([H, F], FP16, name="xt")
            nc.sync.dma_start(out=xt, in_=dram_ap(x_d, c0, 0, RPP), single_packet=True)
            u = up.tile([H, F], FP16, name="u")
            nc.scalar.activation(u, xt, act.Silu)
            w = wp.tile([H, F], FP16, name="w")
            nc.scalar.activation(w, u, act.Tanh, scale=0.5)
            a = ap_.tile([H, F], FP16, name="a")
            o = op_.tile([H, F], FP16, name="o")
            x3 = xt.rearrange("p (m w) -> p m w", w=W)
            u3 = u.rearrange("p (m w) -> p m w", w=W)
            w3 = w.rearrange("p (m w) -> p m w", w=W)
            a3 = a.rearrange("p (m w) -> p m w", w=W)
            o3 = o.rearrange("p (m w) -> p m w", w=W)
            if r == 2:
                # S: out(h) = in(h+1). Rows 0..M-2: same partition, free
                # +128. Row M-1: next partition's row 0 via matmul.
                nc.vector.tensor_tensor(
                    a3[:, 0 : M - 1, :], u3[:, 1:M, :], x3[:, 0 : M - 1, :], AOP.add
                )
                pu = pup.tile([H, W], FP32, name="pu")
                nc.tensor.matmul(
                    pu, pm3[:, 0, :], u3[:, 0:1, :], start=True, stop=True
                )
                nc.vector.tensor_tensor(
                    a3[:, M - 1 : M, :],
                    pu.rearrange("p (q w) -> p q w", q=1),
                    x3[:, M - 1 : M, :],
                    AOP.add,
                )
                # S^2: rows 0..M-3: free +256; rows M-2,M-1: next
                # partition's rows 0,1.
                nc.vector.tensor_tensor(
                    o3[:, 0 : M - 2, :], w3[:, 2:M, :], a3[:, 0 : M - 2, :], AOP.mult
                )
                pw = pwp.tile([H, 2 * W], FP32, name="pw")
                nc.tensor.matmul(
                    pw, pm3[:, 0, :], w3[:, 0:2, :], start=True, stop=True
                )
                nc.vector.tensor_tensor(
                    o3[:, M - 2 : M, :],
                    pw.rearrange("p (q w) -> p q w", q=2),
                    a3[:, M - 2 : M, :],
                    AOP.mult,
                )
            else:
                # S: out(h) = in(h-1). Rows 1..M-1: free -128. Row 0:
                # previous partition's row M-1 via matmul.
                nc.vector.tensor_tensor(
                    a3[:, 1:M, :], u3[:, 0 : M - 1, :], x3[:, 1:M, :], AOP.add
                )
                pu = pup.tile([H, W], FP32, name="pu")
                nc.tensor.matmul(
                    pu, pm3[:, 1, :], u3[:, M - 1 : M, :], start=True, stop=True
                )
                nc.vector.tensor_tensor(
                    a3[:, 0:1, :],
                    pu.rearrange("p (q w) -> p q w", q=1),
                    x3[:, 0:1, :],
                    AOP.add,
                )
                # S^2: rows 2..M-1: free -256; rows 0,1: previous
                # partition's rows M-2,M-1.
                nc.vector.tensor_tensor(
                    o3[:, 2:M, :], w3[:, 0 : M - 2, :], a3[:, 2:M, :], AOP.mult
                )
                pw = pwp.tile([H, 2 * W], FP32, name="pw")
                nc.tensor.matmul(
                    pw, pm3[:, 1, :], w3[:, M - 2 : M, :], start=True, stop=True
                )
                nc.vector.tensor_tensor(
                    o3[:, 0:2, :],
                    pw.rearrange("p (q w) -> p q w", q=2),
                    a3[:, 0:2, :],
                    AOP.mult,
                )
            nc.gpsimd.dma_start(out=dram_ap(o_d, c0, 0, RPP), in_=o, single_packet=True)

        tiles = [(g, r) for g in range(NCHUNK) for r in range(5)]
        last = len(tiles) - 1
        for t, (g, r) in enumerate(tiles):
            c0 = r + 5 * G * g
            if r in (2, 3):
                h_tile(r, c0)
            elif t == 0 or t == last:
                # Taper: two independent half-tiles (shorter pipeline
                # head / tail).
                w_tile(r, c0, 0, RPP // 2)
                w_tile(r, c0, RPP // 2, RPP)
            else:
                w_tile(r, c0, 0, RPP)


_NC_CACHE = {}


def _build():
    key = "nc"
    if key in _NC_CACHE:
        return _NC_CACHE[key]
    nc = bacc.Bacc(
        "TRN2",
        target_bir_lowering=False,
        debug=False,
        enable_asserts=True,
        num_devices=B,
    )
    x_d = nc.dram_tensor("x", [C, H, W], FP16, kind="ExternalInput").ap()
    p_d = nc.dram_tensor("perm", [2, H, H], FP16, kind="ExternalInput").ap()
    o_d = nc.dram_tensor("out", [C, H, W], FP16, kind="ExternalOutput").ap()
    with TileContext(nc) as tc:
        _emit(nc, tc, x_d, o_d, p_d)
    nc.compile()
    _NC_CACHE[key] = nc
    return nc


def _perm_mats():
    pm = np.zeros((2, H, H), dtype=np.float16)
    i = np.arange(H)
    for d, delta in enumerate((1, -1)):
        pm[d, (i // PPC) * PPC + (i % PPC + delta) % PPC, i] = 1.0
    return pm


def run(x, trace=False, tmpdir=None):
    x = np.asarray(x)
    assert x.shape == (B, C, H, W), x.shape
    x16 = np.ascontiguousarray(x, dtype=np.float16)
    nc = _build()
    pm = _perm_mats()
    in_maps = [{"x": x16[i], "perm": pm} for i in range(B)]
    res = run_bass_kernel_spmd(
        nc, in_maps, core_ids=list(range(B)), trace=trace, tmpdir=tmpdir
    )
    half = np.float32(0.5)
    out = np.stack(
        [res.results[i]["out"].astype(np.float32) * half for i in range(B)], axis=0
    )
    return out, res


def kernel(x):
    out, _ = run(x)
    return out


# revision 4
# speedup vs baseline: 1.0291x; 1.0291x over previous
"""Trainium2 Bass kernel for the Ablock spatial paradigm, v7 (fp16).

Reference computation (per sample, C=320 channels of 128x128):
    f    = silu(lem(x))
    fatt = lem(sigmoid(f) - 0.5)
    out  = (f + x) * fatt
where lem applies a per-channel circular 1-pixel shift S_c chosen by c%5:
    0: roll -1 along W   1: roll +1 along W
    2: roll -1 along H   3: roll +1 along H   4: identity

Using sigmoid(z)-0.5 = 0.5*tanh(z/2) and S commuting with elementwise ops:
    u = silu(x);  w = tanh(u/2)
    out = (S u + x) * (0.5 * S^2 w)
The kernel computes o2 = (S u + x) * (S^2 w); the host multiplies by 0.5
during the fp16->fp32 upconversion (exact).

Design (v5):
  - Whole pipeline in fp16 on device (harness gate 2e-2 rel; measured
    pipeline error ~1e-3): halves HBM traffic both ways and enables the
    DVE 2x packed mode (TensorTensor only - ScalarTensorPtr measured 1x
    on HW, so the 0.5 scale moved to the host).
  - ACT is the bottleneck (2 transcendental passes = 68us engine floor at
    1.2GHz): it runs exactly 2 clean full-tile activations per tile, with
    ALL shift handling on DVE input APs (which has slack at fp16 2x).
  - H-shift partition-boundary rows via small PE permutation matmuls into
    PSUM, consumed by DVE.
  - Loads on the sync HWDGE queue; stores on the GpSimd SWDGE queue; the
    permutation-matrix load on the scalar HWDGE queue (so tile 0's load
    is first in the sync queue).
  - Taper: the first and last tiles (W-shift / identity, row-splittable)
    are processed as two independent half-tiles, shortening the pipeline
    head (ACT starts after a half-size load) and tail (half-size final
    multiply + store).

Sharding: pure data-parallel, one batch sample per NeuronCore (B=8).
Layout: G=32 same-shift-type channels per tile [128, 4096] fp16; channel
k of a tile occupies partitions [4k, 4k+4); each partition holds 32
contiguous image rows (8KB). Tile load/store is one 3-dim DMA with 8KB
contiguous runs on both sides.
"""

import numpy as np

import concourse.bacc as bacc
import concourse.mybir as mybir
from concourse.bass_utils import run_bass_kernel_spmd
from concourse.tile import TileContext

B, C, H, W = 8, 320, 128, 128
G = 32  # channels of one shift type per tile
F = G * W  # tile free size (4096)
NTYPE_CH = C // 5  # channels per shift type (64)
NCHUNK = NTYPE_CH // G  # tiles per type (2)
PPC = H // G  # partitions per channel (4)
RPP = H // PPC  # image rows per partition (32)
FP32 = mybir.dt.float32
FP16 = mybir.dt.float16
AOP = mybir.AluOpType


def _emit(nc, tc, x_d, o_d, p_d):
    act = mybir.ActivationFunctionType
    with (
        tc.tile_pool(name="pp", bufs=1) as pp,
        tc.tile_pool(name="xp", bufs=5) as xp,
        tc.tile_pool(name="up", bufs=3) as up,
        tc.tile_pool(name="wp", bufs=3) as wp,
        tc.tile_pool(name="ap", bufs=3) as ap_,
        tc.tile_pool(name="op", bufs=5) as op_,
        tc.tile_pool(name="pup", bufs=2, space="PSUM") as pup,
        tc.tile_pool(name="pwp", bufs=2, space="PSUM") as pwp,
    ):
        # Block-diagonal permutation matrices (32 identical 4x4 circulant
        # blocks): as matmul stationary lhsT, (P_d.T @ v)[p,f] =
        # v[group(p)*4 + (p%4 + d) % 4, f] for d = +1 / -1. Loaded via the
        # scalar HWDGE queue to keep the sync queue free for tile loads.
        pm = pp.tile([H, 2 * H], FP16, name="pm")
        nc.scalar.dma_start(
            out=pm.rearrange("p (d i) -> p d i", d=2),
            in_=p_d.rearrange("d k i -> k d i"),
        )
        pm3 = pm.rearrange("p (d i) -> p d i", d=2)

        M = RPP

        def dram_ap(base, c0, m0, m1):
            ap = (
                base[c0 : c0 + 5 * (G - 1) + 1 : 5]
                .rearrange("k h w -> k (h w)")
                .rearrange("k (q f) -> k q f", f=RPP * W)
            )
            if (m0, m1) != (0, RPP):
                ap = ap[:, :, m0 * W : m1 * W]
            return ap

        def w_tile(r, c0, m0, m1):
            """One (possibly row-sliced) tile of a W-shift (r in {0,1}) or
            identity (r==4) type: rows m0..m1 of each partition."""
            ff = (m1 - m0) * W
            xt = xp.tile([H, ff], FP16, name="xt")
            nc.sync.dma_start(out=xt, in_=dram_ap(x_d, c0, m0, m1), single_packet=True)
            u = up.tile([H, ff], FP16, name="u")
            nc.scalar.activation(u, xt, act.Silu)
            w = wp.tile([H, ff], FP16, name="w")
            nc.scalar.activation(w, u, act.Tanh, scale=0.5)
            a = ap_.tile([H, ff], FP16, name="a")
            o = op_.tile([H, ff], FP16, name="o")
            x3 = xt.rearrange("p (m w) -> p m w", w=W)
            u3 = u.rearrange("p (m w) -> p m w", w=W)
            w3 = w.rearrange("p (m w) -> p m w", w=W)
            a3 = a.rearrange("p (m w) -> p m w", w=W)
            o3 = o.rearrange("p (m w) -> p m w", w=W)
            if r == 0:
                # S: out(.,j) = in(.,j+1) along W
                nc.vector.tensor_tensor(
                    a3[:, :, 0:127], u3[:, :, 1:128], x3[:, :, 0:127], AOP.add
                )
                nc.vector.tensor_tensor(
                    a3[:, :, 127:128], u3[:, :, 0:1], x3[:, :, 127:128], AOP.add
                )
                nc.vector.tensor_tensor(
                    o3[:, :, 0:126], w3[:, :, 2:128], a3[:, :, 0:126], AOP.mult
                )
                nc.vector.tensor_tensor(
                    o3[:, :, 126:128], w3[:, :, 0:2], a3[:, :, 126:128], AOP.mult
                )
            elif r == 1:
                # S: out(.,j) = in(.,j-1) along W
                nc.vector.tensor_tensor(
                    a3[:, :, 1:128], u3[:, :, 0:127], x3[:, :, 1:128], AOP.add
                )
                nc.vector.tensor_tensor(
                    a3[:, :, 0:1], u3[:, :, 127:128], x3[:, :, 0:1], AOP.add
                )
                nc.vector.tensor_tensor(
                    o3[:, :, 2:128], w3[:, :, 0:126], a3[:, :, 2:128], AOP.mult
                )
                nc.vector.tensor_tensor(
                    o3[:, :, 0:2], w3[:, :, 126:128], a3[:, :, 0:2], AOP.mult
                )
            else:
                nc.vector.tensor_tensor(a, u, xt, AOP.add)
                nc.vector.tensor_tensor(o, w, a, AOP.mult)
            nc.gpsimd.dma_start(out=dram_ap(o_d, c0, m0, m1), in_=o, single_packet=True)

        def h_tile(r, c0):
            """One full tile of an H-shift type (r in {2,3})."""
            xt = xp.tile([H, F], FP16, name="xt")
            nc.sync.dma_start(out=xt, in_=dram_ap(x_d, c0, 0, RPP), single_packet=True)
            u = up.tile([H, F], FP16, name="u")
            nc.scalar.activation(u, xt, act.Silu)
            w = wp.tile([H, F], FP16, name="w")
            nc.scalar.activation(w, u, act.Tanh, scale=0.5)
            a = ap_.tile([H, F], FP16, name="a")
            o = op_.tile([H, F], FP16, name="o")
            x3 = xt.rearrange("p (m w) -> p m w", w=W)
            u3 = u.rearrange("p (m w) -> p m w", w=W)
            w3 = w.rearrange("p (m w) -> p m w", w=W)
            a3 = a.rearrange("p (m w) -> p m w", w=W)
            o3 = o.rearrange("p (m w) -> p m w", w=W)
            if r == 2:
                # S: out(h) = in(h+1). Rows 0..M-2: same partition, free
                # +128. Row M-1: next partition's row 0 via matmul.
                nc.vector.tensor_tensor(
                    a3[:, 0 : M - 1, :], u3[:, 1:M, :], x3[:, 0 : M - 1, :], AOP.add
                )
                pu = pup.tile([H, W], FP32, name="pu")
                nc.tensor.matmul(
                    pu, pm3[:, 0, :], u3[:, 0:1, :], start=True, stop=True
                )
                nc.vector.tensor_tensor(
                    a3[:, M - 1 : M, :],
                    pu.rearrange("p (q w) -> p q w", q=1),
                    x3[:, M - 1 : M, :],
                    AOP.add,
                )
                # S^2: rows 0..M-3: free +256; rows M-2,M-1: next
                # partition's rows 0,1.
                nc.vector.tensor_tensor(
                    o3[:, 0 : M - 2, :], w3[:, 2:M, :], a3[:, 0 : M - 2, :], AOP.mult
                )
                pw = pwp.tile([H, 2 * W], FP32, name="pw")
                nc.tensor.matmul(
                    pw, pm3[:, 0, :], w3[:, 0:2, :], start=True, stop=True
                )
                nc.vector.tensor_tensor(
                    o3[:, M - 2 : M, :],
                    pw.rearrange("p (q w) -> p q w", q=2),
                    a3[:, M - 2 : M, :],
                    AOP.mult,
                )
            else:
                # S: out(h) = in(h-1). Rows 1..M-1: free -128. Row 0:
                # previous partition's row M-1 via matmul.
                nc.vector.tensor_tensor(
                    a3[:, 1:M, :], u3[:, 0 : M - 1, :], x3[:, 1:M, :], AOP.add
                )
                pu = pup.tile([H, W], FP32, name="pu")
                nc.tensor.matmul(
                    pu, pm3[:, 1, :], u3[:, M - 1 : M, :], start=True, stop=True
                )
                nc.vector.tensor_tensor(
                    a3[:, 0:1, :],
                    pu.rearrange("p (q w) -> p q w", q=1),
                    x3[:, 0:1, :],
                    AOP.add,
                )
                # S^2: rows 2..M-1: free -256; rows 0,1: previous
                # partition's rows M-2,M-1.
                nc.vector.tensor_tensor(
                    o3[:, 2:M, :], w3[:, 0 : M - 2, :], a3[:, 2:M, :], AOP.mult
                )
                pw = pwp.tile([H, 2 * W], FP32, name="pw")
                nc.tensor.matmul(
                    pw, pm3[:, 1, :], w3[:, M - 2 : M, :], start=True, stop=True
                )
                nc.vector.tensor_tensor(
                    o3[:, 0:2, :],
                    pw.rearrange("p (q w) -> p q w", q=2),
                    a3[:, 0:2, :],
                    AOP.mult,
                )
            nc.gpsimd.dma_start(out=dram_ap(o_d, c0, 0, RPP), in_=o, single_packet=True)

        tiles = [(g, r) for g in range(NCHUNK) for r in range(5)]
        last = len(tiles) - 1
        for t, (g, r) in enumerate(tiles):
            c0 = r + 5 * G * g
            if r in (2, 3):
                h_tile(r, c0)
            elif t == 0 or t == last:
                # Taper: two independent half-tiles (shorter pipeline
                # head / tail).
                w_tile(r, c0, 0, RPP // 2)
                w_tile(r, c0, RPP // 2, RPP)
            else:
                w_tile(r, c0, 0, RPP)


_NC_CACHE = {}


def _build():
    key = "nc"
    if key in _NC_CACHE:
        return _NC_CACHE[key]
    nc = bacc.Bacc(
        "TRN2",
        target_bir_lowering=False,
        debug=False,
        enable_asserts=False,
        num_devices=B,
    )
    x_d = nc.dram_tensor("x", [C, H, W], FP16, kind="ExternalInput").ap()
    p_d = nc.dram_tensor("perm", [2, H, H], FP16, kind="ExternalInput").ap()
    o_d = nc.dram_tensor("out", [C, H, W], FP16, kind="ExternalOutput").ap()
    with TileContext(nc) as tc:
        _emit(nc, tc, x_d, o_d, p_d)
    nc.compile()
    _NC_CACHE[key] = nc
    return nc


def _perm_mats():
    pm = np.zeros((2, H, H), dtype=np.float16)
    i = np.arange(H)
    for d, delta in enumerate((1, -1)):
        pm[d, (i // PPC) * PPC + (i % PPC + delta) % PPC, i] = 1.0
    return pm


def run(x, trace=False, tmpdir=None):
    x = np.asarray(x)
    assert x.shape == (B, C, H, W), x.shape
    x16 = np.ascontiguousarray(x, dtype=np.float16)
    nc = _build()
    pm = _perm_mats()
    in_maps = [{"x": x16[i], "perm": pm} for i in range(B)]
    res = run_bass_kernel_spmd(
        nc, in_maps, core_ids=list(range(B)), trace=trace, tmpdir=tmpdir
    )
    half = np.float32(0.5)
    out = np.stack(
        [res.results[i]["out"].astype(np.float32) * half for i in range(B)], axis=0
    )
    return out, res


def kernel(x):
    out, _ = run(x)
    return out
